# revision 35
# baseline (speedup 1.0000x reference)
"""Trainium2 Bass kernel for nn_AttentiveStylizationBlock (B=8,T=4096,E=1024,M=256,L=512).

Sharding: data-parallel over batch - core i computes batch element i entirely
(weights replicated, no collectives).

Math per batch element (algebraically refactored from the reference):
    k   = latent @ Wk + bk                      [M, E]
    v   = latent @ Wv + bv                      [M, E]
    kq  = Wq @ k^T                              [E, M]   (folds the q-projection:
          w = (emb Wq + bq) k^T = emb . kq + bq . k^T)
    c   = (bq . k^T) / sqrt(E)                  [M]
    ew[m,t] = exp(kq[:,m] . emb[t,:] / sqrt(E) + c[m])
    S[m]    = sum_t ew[m,t]                     (softmax over frames T, dim=1)
    vn  = v / S[:, None]
    pred[t] = sum_m ew[m,t] * vn[m]             [T, E]
    out = LN(pred + emb) * gamma + beta

Implementation notes (v4):
  - All matmul operands are bf16 (psum accumulation stays f32). emb is
    read from HBM exactly once: f32 over the sync HWDGE queue into a
    small staging buffer, cast to a bf16 SBUF stash on gpsimd. Weights
    Wk/Wv/latent ride the gpsimd SWDGE cast-DMA queue concurrently; Wq
    streams through in f32 row-blocks (it is used once, transposed).
    Output stores use the scalar-engine HWDGE queue. Three DMA queues
    run concurrently.
  - The residual add (pred + emb) is done on the PE via an extra
    identity-matmul accumulation into a single [P, 1024] two-bank psum.
  - One scalar-engine activation evacuates x (psum -> SBUF bf16) and
    yields sum_e x via its accumulator; one DVE scalar_tensor_tensor
    squares x and accumulates sum_e x^2. Per-row LN statistics are
    computed batched over pairs of t-blocks; 1/sqrt(var+eps) is a
    single Abs_reciprocal_sqrt activation. The normalize
    (x*rstd - mu*rstd) is split column-wise across gpsimd/ACT/DVE.
  - gamma/beta application is skipped when gamma==1, beta==0 (values
    checked at run time; a general variant is compiled on demand).
"""

import os
import sys

sys.path.insert(0, "/opt/trn_rl_repo")

import numpy as np

B, T, E, M, L = 8, 4096, 1024, 256, 512
P = 128
EPS = 1e-6
ES = E // P        # 8  e-subtiles
LS = L // P        # 4  l-subtiles
MB = M // P        # 2  m-blocks
TT = 512           # t-tile (free dim of the big matmuls)
NT = T // TT       # 8  t-tiles
TS = TT // P       # 4  t-subblocks per t-tile
NTS = T // P       # 32 t-subblocks total
NH = T // 256      # 16 emb staging half-tiles
EH = E // 512      # 2  e-halves (psum free-dim limit for fp32)
SCALE = 1.0 / float(np.sqrt(E))

# norm column split: [ACT | gpsimd | DVE]
NSPLIT = (128, 384, 512)

REPS = int(os.environ.get("KERNEL_REPS", "1"))

_cache = {}
LAST_RUN = {}


def _bcast_ap(ap, p):
    """[free...] DRAM AP -> [p, free...] partition-broadcast AP."""
    import concourse.bass as bass

    return bass.AP(tensor=ap.tensor, offset=ap.offset, ap=[[0, p], *ap.ap])


def _build(apply_affine, reps=None):
    if reps is None:
        reps = REPS
    import concourse.bacc as bacc
    import concourse.mybir as mybir
    import concourse.tile as tile
    from concourse.masks import make_identity

    f32 = mybir.dt.float32
    bf16 = mybir.dt.bfloat16
    AF = mybir.ActivationFunctionType
    OP = mybir.AluOpType
    nc = bacc.Bacc(None, target_bir_lowering=False)

    emb = nc.dram_tensor("emb", (T, E), f32, kind="ExternalInput")
    latent = nc.dram_tensor("latent", (M, L), f32, kind="ExternalInput")
    Wq = nc.dram_tensor("Wq", (E, E), f32, kind="ExternalInput")
    bq = nc.dram_tensor("bq", (E,), f32, kind="ExternalInput")
    Wk = nc.dram_tensor("Wk", (L, E), f32, kind="ExternalInput")
    bk = nc.dram_tensor("bk", (E,), f32, kind="ExternalInput")
    Wv = nc.dram_tensor("Wv", (L, E), f32, kind="ExternalInput")
    bv = nc.dram_tensor("bv", (E,), f32, kind="ExternalInput")
    gamma = nc.dram_tensor("gamma", (E,), f32, kind="ExternalInput")
    beta = nc.dram_tensor("beta", (E,), f32, kind="ExternalInput")
    out = nc.dram_tensor("out", (T, E), f32, kind="ExternalOutput")

    with tile.TileContext(nc) as tc, \
         tc.tile_pool(name="const", bufs=1) as const, \
         tc.tile_pool(name="persist", bufs=1) as persist, \
         tc.tile_pool(name="wload", bufs=2) as wload, \
         tc.tile_pool(name="embtp", bufs=2) as embtp, \
         tc.tile_pool(name="xsb", bufs=4) as xsbp, \
         tc.tile_pool(name="xout", bufs=2) as xoutp, \
         tc.tile_pool(name="small", bufs=4) as small, \
         tc.tile_pool(name="sqscr", bufs=1) as sqscr, \
         tc.tile_pool(name="psum_tr", bufs=2, space="PSUM") as psum_tr, \
         tc.tile_pool(name="psum_mm", bufs=2, space="PSUM") as psum_mm, \
         tc.tile_pool(name="psum_p2", bufs=2, space="PSUM") as psum_p2:

        # ---- constants ----
        f32r = mybir.dt.float32r
        ident = const.tile([P, P], bf16)
        make_identity(nc, ident)
        ident_f = const.tile([P, P], f32)
        make_identity(nc, ident_f)
        bq_bf = const.tile([P, ES], bf16)
        nc.gpsimd.dma_start(out=bq_bf, in_=bq[:].rearrange("(o p) -> p o", p=P))
        bk_pp = const.tile([P, ES], f32)
        nc.sync.dma_start(bk_pp, bk[:].rearrange("(o p) -> p o", p=P))
        bv_bc = const.tile([P, E], f32)
        nc.gpsimd.dma_start(out=bv_bc, in_=_bcast_ap(bv[:], P))
        if apply_affine:
            gamma_bc = const.tile([P, E], f32)
            nc.gpsimd.dma_start(out=gamma_bc, in_=_bcast_ap(gamma[:], P))
            beta_bc = const.tile([P, E], f32)
            nc.gpsimd.dma_start(out=beta_bc, in_=_bcast_ap(beta[:], P))
        eps_c = const.tile([P, 1], f32)
        nc.vector.memset(eps_c, EPS)

        for _rep in range(reps):
            # ---- SWDGE cast-DMAs: latent, Wk, Wv ----
            lat_bf = persist.tile([P, MB, L], bf16, tag="latbf")
            nc.gpsimd.dma_start(
                out=lat_bf, in_=latent[:, :].rearrange("(mb p) l -> p mb l", p=P))
            wk_bf = persist.tile([P, LS, E], bf16, tag="wkbf")
            nc.gpsimd.dma_start(
                out=wk_bf, in_=Wk[:, :].rearrange("(lo p) e -> p lo e", p=P))

            # ---- sync HWDGE: Wq f32 (used once, transposed in f32) ----
            wq_f = persist.tile([P, ES, E], f32, tag="wqf32")
            nc.sync.dma_start(
                wq_f, Wq[:, :].rearrange("(eb p) e -> p eb e", p=P))

            # ---- emb: SWDGE cast-DMA into the bf16 stash, one tile/512 t ----
            emb_bf = persist.tile([P, NTS, E], bf16, tag="embbf")

            def emb_tile_dma(it):
                nc.gpsimd.dma_start(
                    out=emb_bf[:, it * TS:(it + 1) * TS, :],
                    in_=emb[it * TT:(it + 1) * TT, :].rearrange(
                        "(ts p) e -> p ts e", p=P))

            emb_tile_dma(0)
            emb_tile_dma(1)
            wv_bf = persist.tile([P, LS, E], bf16, tag="wvbf")
            nc.gpsimd.dma_start(
                out=wv_bf, in_=Wv[:, :].rearrange("(lo p) e -> p lo e", p=P))
            for it in range(2, NT):
                emb_tile_dma(it)

            # ---- latent^T  [l, m] ----
            latT = persist.tile([P, LS, M], bf16, tag="latT")
            for mb in range(MB):
                pst = psum_tr.tile([P, TT], bf16, tag="tr")
                for ls in range(LS):
                    nc.tensor.transpose(pst[:, ls * P:(ls + 1) * P],
                                        lat_bf[:, mb, ls * P:(ls + 1) * P], ident)
                nc.vector.tensor_copy(
                    out=latT[:, :, mb * P:(mb + 1) * P],
                    in_=pst.rearrange("p (ls m) -> p ls m", ls=LS))

            # ---- k^T [e, m] = Wk^T latT (+bk), f32r (feeds f32r matmuls) ----
            k_f = persist.tile([P, ES, M], f32r, tag="kf")
            k_bf = persist.tile([P, ES, M], bf16, tag="kbf")
            for es in range(ES):
                ps = psum_mm.tile([P, 512], f32, tag="mm")
                for ls in range(LS):
                    nc.tensor.matmul(ps[:, :M], wk_bf[:, ls, es * P:(es + 1) * P],
                                     latT[:, ls, :],
                                     start=(ls == 0), stop=(ls == LS - 1))
                nc.scalar.activation(k_f[:, es, :], ps[:, :M], AF.Identity,
                                     bias=bk_pp[:, es:es + 1])
                nc.vector.tensor_copy(out=k_bf[:, es, :], in_=k_f[:, es, :])

            # ---- c [m] = (bq . k^T) * SCALE  (f32r) ----
            c_pp = const.tile([P, MB], f32)
            for mb in range(MB):
                ps = psum_mm.tile([P, 512], f32, tag="mm")
                for es in range(ES):
                    nc.tensor.matmul(
                        ps[:, :1],
                        k_bf[:, es, mb * P:(mb + 1) * P],
                        bq_bf[:, es:es + 1],
                        start=(es == 0), stop=(es == ES - 1))
                nc.scalar.mul(c_pp[:, mb:mb + 1], ps[:, :1], SCALE)

            # ---- kq [e_in, m] = Wq @ k^T (f32 transposes, f32r matmuls) ----
            kq = persist.tile([P, ES, M], bf16, tag="kq")
            for eb in range(ES):
                wqT_row = wload.tile([P, ES, P], f32r, tag="wqTrow")
                for half in range(2):
                    pst = psum_mm.tile([P, TT], f32, tag="mm")
                    for j in range(4):
                        fs = half * 4 + j
                        nc.tensor.transpose(pst[:, j * P:(j + 1) * P],
                                            wq_f[:, eb, fs * P:(fs + 1) * P],
                                            ident_f)
                    nc.vector.tensor_copy(
                        out=wqT_row[:, half * 4:(half + 1) * 4, :],
                        in_=pst.rearrange("p (j m) -> p j m", j=4))
                ps = psum_mm.tile([P, 512], f32, tag="mm")
                for fs in range(ES):
                    nc.tensor.matmul(ps[:, :M],
                                     wqT_row[:, fs, :],
                                     k_f[:, fs, :],
                                     start=(fs == 0), stop=(fs == ES - 1))
                nc.scalar.copy(kq[:, eb, :], ps[:, :M])

            # ---- v [m, e] = latT^T Wv + bv (unnormalized) ----
            v_bf = persist.tile([P, MB, E], bf16, tag="vbf")
            for mb in range(MB):
                for eh in range(EH):
                    ps = psum_mm.tile([P, 512], f32, tag="mm")
                    for ls in range(LS):
                        nc.tensor.matmul(ps, latT[:, ls, mb * P:(mb + 1) * P],
                                         wv_bf[:, ls, eh * 512:(eh + 1) * 512],
                                         start=(ls == 0), stop=(ls == LS - 1))
                    nc.vector.scalar_tensor_tensor(
                        out=v_bf[:, mb, eh * 512:(eh + 1) * 512],
                        in0=ps, scalar=1.0,
                        in1=bv_bc[:, eh * 512:(eh + 1) * 512],
                        op0=OP.mult, op1=OP.add)

            # ---- pass 1 over T: exp_wT [m, t] and row sums ----
            exp_wT = persist.tile([P, MB, T], bf16, tag="expw")
            s_part = persist.tile([P, MB, NT], f32, tag="spart")
            for it in range(NT):
                embT = embtp.tile([P, ES, TT], bf16, tag="embT")
                for es in range(ES):
                    pst = psum_tr.tile([P, TT], bf16, tag="tr")
                    for ts in range(TS):
                        nc.tensor.transpose(
                            pst[:, ts * P:(ts + 1) * P],
                            emb_bf[:, it * TS + ts, es * P:(es + 1) * P], ident)
                    if es % 4 == 3:
                        nc.scalar.copy(embT[:, es, :], pst)
                    else:
                        nc.vector.tensor_copy(out=embT[:, es, :], in_=pst)
                for mb in range(MB):
                    psw = psum_mm.tile([P, 512], f32, tag="mm")
                    for es in range(ES):
                        nc.tensor.matmul(psw, kq[:, es, mb * P:(mb + 1) * P],
                                         embT[:, es, :],
                                         start=(es == 0), stop=(es == ES - 1))
                    nc.scalar.activation(exp_wT[:, mb, it * TT:(it + 1) * TT], psw,
                                         AF.Exp, bias=c_pp[:, mb:mb + 1], scale=SCALE,
                                         accum_out=s_part[:, mb, it:it + 1])

            # ---- softmax denominators and normalized v ----
            s_tot = small.tile([P, MB, 1], f32, tag="stot")
            nc.vector.reduce_sum(s_tot, s_part, axis=mybir.AxisListType.X)
            inv_s = small.tile([P, MB, 1], f32, tag="invs")
            nc.vector.reciprocal(inv_s, s_tot)
            v_norm = persist.tile([P, MB, E], bf16, tag="vnorm")
            for mb in range(MB):
                nc.vector.tensor_scalar_mul(v_norm[:, mb, :], v_bf[:, mb, :],
                                            inv_s[:, mb, :])

            # ---- pass 2 over T: pred + residual + LayerNorm, fused ----
            n0, n1, n2 = NSPLIT
            for tsb in range(NTS):
                t0 = tsb * P
                psp = psum_p2.tile([P, 1024], f32, tag="p2")
                for eh in range(EH):
                    sl = psp[:, eh * 512:(eh + 1) * 512]
                    for mb in range(MB):
                        nc.tensor.matmul(sl, exp_wT[:, mb, t0:t0 + P],
                                         v_norm[:, mb, eh * 512:(eh + 1) * 512],
                                         start=(mb == 0), stop=False)
                    nc.tensor.matmul(sl, ident,
                                     emb_bf[:, tsb, eh * 512:(eh + 1) * 512],
                                     start=False, stop=True)
                # evacuate x (bf16) + sum_e x in one scalar-engine op
                x_sb = xsbp.tile([P, E], bf16, tag="xsb")
                sx = small.tile([P, 1], f32, tag="sx")
                nc.scalar.activation(x_sb, psp, AF.Copy, accum_out=sx)
                # sum_e x^2 on the DVE: out=(x*1)*x with accum=sum
                sqs = sqscr.tile([P, E], bf16, tag="sqscratch")
                ssq = small.tile([P, 1], f32, tag="ssq")
                nc.vector.scalar_tensor_tensor(
                    out=sqs, in0=x_sb, scalar=1.0, in1=x_sb,
                    op0=OP.mult, op1=OP.mult, accum_out=ssq)
                # LN stats: mu (gpsimd), mu^2 (ACT), var+eps (DVE),
                # rstd = 1/sqrt(var+eps) (ACT), -mu*rstd (DVE)
                mu = small.tile([P, 1], f32, tag="mu")
                nc.gpsimd.tensor_scalar_mul(mu, sx, 1.0 / E)
                musq = small.tile([P, 1], f32, tag="musq")
                nc.scalar.activation(musq, mu, AF.Square)
                veps = small.tile([P, 1], f32, tag="veps")
                nc.vector.scalar_tensor_tensor(
                    out=veps, in0=ssq, scalar=1.0 / E, in1=musq,
                    op0=OP.mult, op1=OP.subtract)
                rstd = small.tile([P, 1], f32, tag="rstd")
                nc.scalar.activation(rstd, veps, AF.Abs_reciprocal_sqrt,
                                     bias=eps_c)
                nmr = small.tile([P, 1], f32, tag="nmr")
                nc.vector.scalar_tensor_tensor(
                    out=nmr, in0=mu, scalar=-1.0, in1=rstd,
                    op0=OP.mult, op1=OP.mult)
                # out = x*rstd - mu*rstd, columns split ACT / gpsimd / DVE
                xo = xoutp.tile([P, E], f32, tag="xo")
                nc.scalar.activation(xo[:, 0:n0], x_sb[:, 0:n0],
                                     AF.Identity, bias=nmr, scale=rstd)
                nc.gpsimd.tensor_scalar(xo[:, n0:n0 + n1],
                                        in0=x_sb[:, n0:n0 + n1],
                                        scalar1=rstd, scalar2=nmr,
                                        op0=OP.mult, op1=OP.add)
                nc.vector.tensor_scalar(xo[:, n0 + n1:E],
                                        in0=x_sb[:, n0 + n1:E],
                                        scalar1=rstd, scalar2=nmr,
                                        op0=OP.mult, op1=OP.add)
                if apply_affine:
                    nc.vector.tensor_mul(xo, xo, gamma_bc)
                    nc.gpsimd.tensor_add(xo, xo, beta_bc)
                nc.sync.dma_start(out[t0:t0 + P, :], xo)

    nc.compile()
    return nc


def kernel(emb, latent, Wq, bq, Wk, bk, Wv, bv, gamma, beta):
    from concourse.bass_utils import run_bass_kernel_spmd

    gamma = np.ascontiguousarray(gamma, dtype=np.float32)
    beta = np.ascontiguousarray(beta, dtype=np.float32)
    apply_affine = not (np.all(gamma == 1.0) and np.all(beta == 0.0))

    key = ("nc", apply_affine)
    if key not in _cache:
        _cache[key] = _build(apply_affine)
    nc = _cache[key]

    emb = np.ascontiguousarray(emb, dtype=np.float32)
    latent = np.ascontiguousarray(latent, dtype=np.float32)
    shared = {
        "Wq": np.ascontiguousarray(Wq, dtype=np.float32),
        "bq": np.ascontiguousarray(bq, dtype=np.float32),
        "Wk": np.ascontiguousarray(Wk, dtype=np.float32),
        "bk": np.ascontiguousarray(bk, dtype=np.float32),
        "Wv": np.ascontiguousarray(Wv, dtype=np.float32),
        "bv": np.ascontiguousarray(bv, dtype=np.float32),
        "gamma": gamma,
        "beta": beta,
    }
    in_maps = [
        {"emb": emb[b], "latent": latent[b], **shared} for b in range(B)
    ]
    trace = bool(int(os.environ.get("KERNEL_TRACE", "0")))
    res = run_bass_kernel_spmd(nc, in_maps, list(range(B)), trace=trace)
    LAST_RUN["exec_time_ns"] = res.exec_time_ns
    LAST_RUN["mean_exec_time_ns"] = res.mean_exec_time_ns
    LAST_RUN["profile_json"] = res.profile_json
    return np.stack([res.results[b]["out"] for b in range(B)], axis=0)


# revision 36
# speedup vs baseline: 1.0973x; 1.0973x over previous
"""Trainium2 Bass kernel for nn_AttentiveStylizationBlock (B=8,T=4096,E=1024,M=256,L=512).

Sharding: data-parallel over batch - core i computes batch element i entirely
(weights replicated, no collectives).

Math per batch element (algebraically refactored from the reference):
    k   = latent @ Wk + bk                      [M, E]
    v   = latent @ Wv + bv                      [M, E]
    kq  = Wq @ k^T                              [E, M]   (folds the q-projection:
          w = (emb Wq + bq) k^T = emb . kq + bq . k^T)
    c   = (bq . k^T) / sqrt(E)                  [M]
    ew[m,t] = exp(kq[:,m] . emb[t,:] / sqrt(E) + c[m])
    S[m]    = sum_t ew[m,t]                     (softmax over frames T, dim=1)
    vn  = v / S[:, None]
    pred[t] = sum_m ew[m,t] * vn[m]             [T, E]
    out = LN(pred + emb) * gamma + beta

Implementation notes (v4):
  - All matmul operands are bf16 (psum accumulation stays f32). emb is
    read from HBM exactly once: f32 over the sync HWDGE queue into a
    small staging buffer, cast to a bf16 SBUF stash on gpsimd. Weights
    Wk/Wv/latent ride the gpsimd SWDGE cast-DMA queue concurrently; Wq
    streams through in f32 row-blocks (it is used once, transposed).
    Output stores use the scalar-engine HWDGE queue. Three DMA queues
    run concurrently.
  - The residual add (pred + emb) is done on the PE via an extra
    identity-matmul accumulation into a single [P, 1024] two-bank psum.
  - One scalar-engine activation evacuates x (psum -> SBUF bf16) and
    yields sum_e x via its accumulator; one DVE scalar_tensor_tensor
    squares x and accumulates sum_e x^2. Per-row LN statistics are
    computed batched over pairs of t-blocks; 1/sqrt(var+eps) is a
    single Abs_reciprocal_sqrt activation. The normalize
    (x*rstd - mu*rstd) is split column-wise across gpsimd/ACT/DVE.
  - gamma/beta application is skipped when gamma==1, beta==0 (values
    checked at run time; a general variant is compiled on demand).
"""

import os
import sys

sys.path.insert(0, "/opt/trn_rl_repo")

import numpy as np

B, T, E, M, L = 8, 4096, 1024, 256, 512
P = 128
EPS = 1e-6
ES = E // P        # 8  e-subtiles
LS = L // P        # 4  l-subtiles
MB = M // P        # 2  m-blocks
TT = 512           # t-tile (free dim of the big matmuls)
NT = T // TT       # 8  t-tiles
TS = TT // P       # 4  t-subblocks per t-tile
NTS = T // P       # 32 t-subblocks total
NH = T // 256      # 16 emb staging half-tiles
EH = E // 512      # 2  e-halves (psum free-dim limit for fp32)
SCALE = 1.0 / float(np.sqrt(E))

# norm column split: [ACT | gpsimd | DVE]
NSPLIT = (384, 512, 128)

REPS = int(os.environ.get("KERNEL_REPS", "1"))

_cache = {}
LAST_RUN = {}


def _bcast_ap(ap, p):
    """[free...] DRAM AP -> [p, free...] partition-broadcast AP."""
    import concourse.bass as bass

    return bass.AP(tensor=ap.tensor, offset=ap.offset, ap=[[0, p], *ap.ap])


def _build(apply_affine, reps=None):
    if reps is None:
        reps = REPS
    import concourse.bacc as bacc
    import concourse.mybir as mybir
    import concourse.tile as tile
    from concourse.masks import make_identity

    f32 = mybir.dt.float32
    bf16 = mybir.dt.bfloat16
    AF = mybir.ActivationFunctionType
    OP = mybir.AluOpType
    nc = bacc.Bacc(None, target_bir_lowering=False)

    emb = nc.dram_tensor("emb", (T, E), f32, kind="ExternalInput")
    latent = nc.dram_tensor("latent", (M, L), f32, kind="ExternalInput")
    Wq = nc.dram_tensor("Wq", (E, E), f32, kind="ExternalInput")
    bq = nc.dram_tensor("bq", (E,), f32, kind="ExternalInput")
    Wk = nc.dram_tensor("Wk", (L, E), f32, kind="ExternalInput")
    bk = nc.dram_tensor("bk", (E,), f32, kind="ExternalInput")
    Wv = nc.dram_tensor("Wv", (L, E), f32, kind="ExternalInput")
    bv = nc.dram_tensor("bv", (E,), f32, kind="ExternalInput")
    gamma = nc.dram_tensor("gamma", (E,), f32, kind="ExternalInput")
    beta = nc.dram_tensor("beta", (E,), f32, kind="ExternalInput")
    out = nc.dram_tensor("out", (T, E), f32, kind="ExternalOutput")

    with tile.TileContext(nc) as tc, \
         tc.tile_pool(name="const", bufs=1) as const, \
         tc.tile_pool(name="persist", bufs=1) as persist, \
         tc.tile_pool(name="wload", bufs=2) as wload, \
         tc.tile_pool(name="embtp", bufs=2) as embtp, \
         tc.tile_pool(name="xsb", bufs=4) as xsbp, \
         tc.tile_pool(name="xout", bufs=2) as xoutp, \
         tc.tile_pool(name="small", bufs=6) as small, \
         tc.tile_pool(name="sqscr", bufs=1) as sqscr, \
         tc.tile_pool(name="psum_tr", bufs=2, space="PSUM") as psum_tr, \
         tc.tile_pool(name="psum_mm", bufs=2, space="PSUM") as psum_mm, \
         tc.tile_pool(name="psum_p2", bufs=2, space="PSUM") as psum_p2:

        # ---- constants ----
        f32r = mybir.dt.float32r
        ident = const.tile([P, P], bf16)
        make_identity(nc, ident)
        ident_f = const.tile([P, P], f32)
        make_identity(nc, ident_f)
        bq_bf = const.tile([P, ES], bf16)
        nc.gpsimd.dma_start(out=bq_bf, in_=bq[:].rearrange("(o p) -> p o", p=P))
        bk_pp = const.tile([P, ES], f32)
        nc.sync.dma_start(bk_pp, bk[:].rearrange("(o p) -> p o", p=P))
        bv_bc = const.tile([P, E], f32)
        nc.gpsimd.dma_start(out=bv_bc, in_=_bcast_ap(bv[:], P))
        if apply_affine:
            gamma_bc = const.tile([P, E], f32)
            nc.gpsimd.dma_start(out=gamma_bc, in_=_bcast_ap(gamma[:], P))
            beta_bc = const.tile([P, E], f32)
            nc.gpsimd.dma_start(out=beta_bc, in_=_bcast_ap(beta[:], P))
        eps_c = const.tile([P, 1], f32)
        nc.vector.memset(eps_c, EPS)

        for _rep in range(reps):
            # ---- SWDGE cast-DMAs: latent, Wk, Wv ----
            lat_bf = persist.tile([P, MB, L], bf16, tag="latbf")
            nc.gpsimd.dma_start(
                out=lat_bf, in_=latent[:, :].rearrange("(mb p) l -> p mb l", p=P))
            wk_bf = persist.tile([P, LS, E], bf16, tag="wkbf")
            nc.gpsimd.dma_start(
                out=wk_bf, in_=Wk[:, :].rearrange("(lo p) e -> p lo e", p=P))

            # ---- sync HWDGE: Wq f32 (used once, transposed in f32) ----
            wq_f = persist.tile([P, ES, E], f32, tag="wqf32")
            nc.sync.dma_start(
                wq_f, Wq[:, :].rearrange("(eb p) e -> p eb e", p=P))

            # ---- emb: SWDGE cast-DMA into the bf16 stash, one tile/512 t ----
            emb_bf = persist.tile([P, NTS, E], bf16, tag="embbf")

            def emb_tile_dma(it):
                nc.gpsimd.dma_start(
                    out=emb_bf[:, it * TS:(it + 1) * TS, :],
                    in_=emb[it * TT:(it + 1) * TT, :].rearrange(
                        "(ts p) e -> p ts e", p=P))

            emb_tile_dma(0)
            emb_tile_dma(1)
            wv_bf = persist.tile([P, LS, E], bf16, tag="wvbf")
            nc.gpsimd.dma_start(
                out=wv_bf, in_=Wv[:, :].rearrange("(lo p) e -> p lo e", p=P))
            for it in range(2, NT):
                emb_tile_dma(it)

            # ---- latent^T  [l, m] ----
            latT = persist.tile([P, LS, M], bf16, tag="latT")
            for mb in range(MB):
                pst = psum_tr.tile([P, TT], bf16, tag="tr")
                for ls in range(LS):
                    nc.tensor.transpose(pst[:, ls * P:(ls + 1) * P],
                                        lat_bf[:, mb, ls * P:(ls + 1) * P], ident)
                nc.vector.tensor_copy(
                    out=latT[:, :, mb * P:(mb + 1) * P],
                    in_=pst.rearrange("p (ls m) -> p ls m", ls=LS))

            # ---- k^T [e, m] = Wk^T latT (+bk), f32r (feeds f32r matmuls) ----
            k_f = persist.tile([P, ES, M], f32r, tag="kf")
            k_bf = persist.tile([P, ES, M], bf16, tag="kbf")
            for es in range(ES):
                ps = psum_mm.tile([P, 512], f32, tag="mm")
                for ls in range(LS):
                    nc.tensor.matmul(ps[:, :M], wk_bf[:, ls, es * P:(es + 1) * P],
                                     latT[:, ls, :],
                                     start=(ls == 0), stop=(ls == LS - 1))
                nc.scalar.activation(k_f[:, es, :], ps[:, :M], AF.Identity,
                                     bias=bk_pp[:, es:es + 1])
                nc.vector.tensor_copy(out=k_bf[:, es, :], in_=k_f[:, es, :])

            # ---- c [m] = (bq . k^T) * SCALE  (f32r) ----
            c_pp = const.tile([P, MB], f32)
            for mb in range(MB):
                ps = psum_mm.tile([P, 512], f32, tag="mm")
                for es in range(ES):
                    nc.tensor.matmul(
                        ps[:, :1],
                        k_bf[:, es, mb * P:(mb + 1) * P],
                        bq_bf[:, es:es + 1],
                        start=(es == 0), stop=(es == ES - 1))
                nc.scalar.mul(c_pp[:, mb:mb + 1], ps[:, :1], SCALE)

            # ---- kq [e_in, m] = Wq @ k^T (f32 transposes, f32r matmuls) ----
            kq = persist.tile([P, ES, M], bf16, tag="kq")
            for eb in range(ES):
                wqT_row = wload.tile([P, ES, P], f32r, tag="wqTrow")
                for half in range(2):
                    pst = psum_mm.tile([P, TT], f32, tag="mm")
                    for j in range(4):
                        fs = half * 4 + j
                        nc.tensor.transpose(pst[:, j * P:(j + 1) * P],
                                            wq_f[:, eb, fs * P:(fs + 1) * P],
                                            ident_f)
                    nc.vector.tensor_copy(
                        out=wqT_row[:, half * 4:(half + 1) * 4, :],
                        in_=pst.rearrange("p (j m) -> p j m", j=4))
                ps = psum_mm.tile([P, 512], f32, tag="mm")
                for fs in range(ES):
                    nc.tensor.matmul(ps[:, :M],
                                     wqT_row[:, fs, :],
                                     k_f[:, fs, :],
                                     start=(fs == 0), stop=(fs == ES - 1))
                nc.scalar.copy(kq[:, eb, :], ps[:, :M])

            # ---- v [m, e] = latT^T Wv + bv (unnormalized) ----
            v_bf = persist.tile([P, MB, E], bf16, tag="vbf")
            for mb in range(MB):
                for eh in range(EH):
                    ps = psum_mm.tile([P, 512], f32, tag="mm")
                    for ls in range(LS):
                        nc.tensor.matmul(ps, latT[:, ls, mb * P:(mb + 1) * P],
                                         wv_bf[:, ls, eh * 512:(eh + 1) * 512],
                                         start=(ls == 0), stop=(ls == LS - 1))
                    nc.vector.scalar_tensor_tensor(
                        out=v_bf[:, mb, eh * 512:(eh + 1) * 512],
                        in0=ps, scalar=1.0,
                        in1=bv_bc[:, eh * 512:(eh + 1) * 512],
                        op0=OP.mult, op1=OP.add)

            # ---- pass 1 over T: exp_wT [m, t] and row sums ----
            exp_wT = persist.tile([P, MB, T], bf16, tag="expw")
            s_part = persist.tile([P, MB, NT], f32, tag="spart")
            for it in range(NT):
                embT = embtp.tile([P, ES, TT], bf16, tag="embT")
                for es in range(ES):
                    pst = psum_tr.tile([P, TT], bf16, tag="tr")
                    for ts in range(TS):
                        nc.tensor.transpose(
                            pst[:, ts * P:(ts + 1) * P],
                            emb_bf[:, it * TS + ts, es * P:(es + 1) * P], ident)
                    if es % 4 == 3:
                        nc.scalar.copy(embT[:, es, :], pst)
                    else:
                        nc.vector.tensor_copy(out=embT[:, es, :], in_=pst)
                for mb in range(MB):
                    psw = psum_mm.tile([P, 512], f32, tag="mm")
                    for es in range(ES):
                        nc.tensor.matmul(psw, kq[:, es, mb * P:(mb + 1) * P],
                                         embT[:, es, :],
                                         start=(es == 0), stop=(es == ES - 1))
                    nc.scalar.activation(exp_wT[:, mb, it * TT:(it + 1) * TT], psw,
                                         AF.Exp, bias=c_pp[:, mb:mb + 1], scale=SCALE,
                                         accum_out=s_part[:, mb, it:it + 1])

            # ---- softmax denominators and normalized v ----
            s_tot = small.tile([P, MB, 1], f32, tag="stot")
            nc.vector.reduce_sum(s_tot, s_part, axis=mybir.AxisListType.X)
            inv_s = small.tile([P, MB, 1], f32, tag="invs")
            nc.vector.reciprocal(inv_s, s_tot)
            v_norm = persist.tile([P, MB, E], bf16, tag="vnorm")
            for mb in range(MB):
                nc.vector.tensor_scalar_mul(v_norm[:, mb, :], v_bf[:, mb, :],
                                            inv_s[:, mb, :])

            # ---- pass 2 over T: pred + residual + LayerNorm, fused ----
            n0, n1, n2 = NSPLIT
            for tsb in range(NTS):
                t0 = tsb * P
                psp = psum_p2.tile([P, 1024], f32, tag="p2")
                for eh in range(EH):
                    sl = psp[:, eh * 512:(eh + 1) * 512]
                    for mb in range(MB):
                        nc.tensor.matmul(sl, exp_wT[:, mb, t0:t0 + P],
                                         v_norm[:, mb, eh * 512:(eh + 1) * 512],
                                         start=(mb == 0), stop=False)
                    nc.tensor.matmul(sl, ident,
                                     emb_bf[:, tsb, eh * 512:(eh + 1) * 512],
                                     start=False, stop=True)
                # evacuate x (bf16) + sum_e x in one scalar-engine op
                x_sb = xsbp.tile([P, E], bf16, tag="xsb")
                sx = small.tile([P, 1], f32, tag="sx")
                nc.scalar.activation(x_sb, psp, AF.Copy, accum_out=sx)
                # sum_e x^2 on the DVE: out=(x*1)*x with accum=sum
                sqs = sqscr.tile([P, E], bf16, tag="sqscratch")
                ssq = small.tile([P, 1], f32, tag="ssq")
                nc.vector.scalar_tensor_tensor(
                    out=sqs, in0=x_sb, scalar=1.0, in1=x_sb,
                    op0=OP.mult, op1=OP.mult, accum_out=ssq)
                # LN stats, all-DVE chain (no cross-engine hops):
                # nmusq = -(sx*sx)/E^2 ; veps = ssq/E + nmusq ;
                # rstd = 1/sqrt(|veps|+eps) (ACT) ; nmr = -sx*rstd/E
                nmusq = small.tile([P, 1], f32, tag="nmusq")
                nc.vector.tensor_scalar(nmusq, in0=sx, scalar1=sx,
                                        scalar2=-1.0 / (E * E),
                                        op0=OP.mult, op1=OP.mult)
                veps = small.tile([P, 1], f32, tag="veps")
                nc.vector.scalar_tensor_tensor(
                    out=veps, in0=ssq, scalar=1.0 / E, in1=nmusq,
                    op0=OP.mult, op1=OP.add)
                rstd = small.tile([P, 1], f32, tag="rstd")
                nc.scalar.activation(rstd, veps, AF.Abs_reciprocal_sqrt,
                                     bias=eps_c)
                nmr = small.tile([P, 1], f32, tag="nmr")
                nc.vector.tensor_scalar(nmr, in0=sx, scalar1=rstd,
                                        scalar2=-1.0 / E,
                                        op0=OP.mult, op1=OP.mult)
                # out = x*rstd - mu*rstd, columns split ACT / gpsimd / DVE
                xo = xoutp.tile([P, E], f32, tag="xo")
                nc.scalar.activation(xo[:, 0:n0], x_sb[:, 0:n0],
                                     AF.Identity, bias=nmr, scale=rstd)
                nc.gpsimd.tensor_scalar(xo[:, n0:n0 + n1],
                                        in0=x_sb[:, n0:n0 + n1],
                                        scalar1=rstd, scalar2=nmr,
                                        op0=OP.mult, op1=OP.add)
                nc.vector.tensor_scalar(xo[:, n0 + n1:E],
                                        in0=x_sb[:, n0 + n1:E],
                                        scalar1=rstd, scalar2=nmr,
                                        op0=OP.mult, op1=OP.add)
                if apply_affine:
                    nc.vector.tensor_mul(xo, xo, gamma_bc)
                    nc.gpsimd.tensor_add(xo, xo, beta_bc)
                nc.sync.dma_start(out[t0:t0 + P, :], xo)

    nc.compile()
    return nc


def kernel(emb, latent, Wq, bq, Wk, bk, Wv, bv, gamma, beta):
    from concourse.bass_utils import run_bass_kernel_spmd

    gamma = np.ascontiguousarray(gamma, dtype=np.float32)
    beta = np.ascontiguousarray(beta, dtype=np.float32)
    apply_affine = not (np.all(gamma == 1.0) and np.all(beta == 0.0))

    key = ("nc", apply_affine)
    if key not in _cache:
        _cache[key] = _build(apply_affine)
    nc = _cache[key]

    emb = np.ascontiguousarray(emb, dtype=np.float32)
    latent = np.ascontiguousarray(latent, dtype=np.float32)
    shared = {
        "Wq": np.ascontiguousarray(Wq, dtype=np.float32),
        "bq": np.ascontiguousarray(bq, dtype=np.float32),
        "Wk": np.ascontiguousarray(Wk, dtype=np.float32),
        "bk": np.ascontiguousarray(bk, dtype=np.float32),
        "Wv": np.ascontiguousarray(Wv, dtype=np.float32),
        "bv": np.ascontiguousarray(bv, dtype=np.float32),
        "gamma": gamma,
        "beta": beta,
    }
    in_maps = [
        {"emb": emb[b], "latent": latent[b], **shared} for b in range(B)
    ]
    trace = bool(int(os.environ.get("KERNEL_TRACE", "0")))
    res = run_bass_kernel_spmd(nc, in_maps, list(range(B)), trace=trace)
    LAST_RUN["exec_time_ns"] = res.exec_time_ns
    LAST_RUN["mean_exec_time_ns"] = res.mean_exec_time_ns
    LAST_RUN["profile_json"] = res.profile_json
    return np.stack([res.results[b]["out"] for b in range(B)], axis=0)


# revision 37
# speedup vs baseline: 1.1384x; 1.0375x over previous
"""Trainium2 Bass kernel for nn_AttentiveStylizationBlock (B=8,T=4096,E=1024,M=256,L=512).

Sharding: data-parallel over batch - core i computes batch element i entirely
(weights replicated, no collectives).

Math per batch element (algebraically refactored from the reference):
    k   = latent @ Wk + bk                      [M, E]
    v   = latent @ Wv + bv                      [M, E]
    kq  = Wq @ k^T                              [E, M]   (folds the q-projection:
          w = (emb Wq + bq) k^T = emb . kq + bq . k^T)
    c   = (bq . k^T) / sqrt(E)                  [M]
    ew[m,t] = exp(kq[:,m] . emb[t,:] / sqrt(E) + c[m])
    S[m]    = sum_t ew[m,t]                     (softmax over frames T, dim=1)
    vn  = v / S[:, None]
    pred[t] = sum_m ew[m,t] * vn[m]             [T, E]
    out = LN(pred + emb) * gamma + beta

Implementation notes (v4):
  - All matmul operands are bf16 (psum accumulation stays f32). emb is
    read from HBM exactly once: f32 over the sync HWDGE queue into a
    small staging buffer, cast to a bf16 SBUF stash on gpsimd. Weights
    Wk/Wv/latent ride the gpsimd SWDGE cast-DMA queue concurrently; Wq
    streams through in f32 row-blocks (it is used once, transposed).
    Output stores use the scalar-engine HWDGE queue. Three DMA queues
    run concurrently.
  - The residual add (pred + emb) is done on the PE via an extra
    identity-matmul accumulation into a single [P, 1024] two-bank psum.
  - One scalar-engine activation evacuates x (psum -> SBUF bf16) and
    yields sum_e x via its accumulator; one DVE scalar_tensor_tensor
    squares x and accumulates sum_e x^2. Per-row LN statistics are
    computed batched over pairs of t-blocks; 1/sqrt(var+eps) is a
    single Abs_reciprocal_sqrt activation. The normalize
    (x*rstd - mu*rstd) is split column-wise across gpsimd/ACT/DVE.
  - gamma/beta application is skipped when gamma==1, beta==0 (values
    checked at run time; a general variant is compiled on demand).
"""

import os
import sys

sys.path.insert(0, "/opt/trn_rl_repo")

import numpy as np

B, T, E, M, L = 8, 4096, 1024, 256, 512
P = 128
EPS = 1e-6
ES = E // P        # 8  e-subtiles
LS = L // P        # 4  l-subtiles
MB = M // P        # 2  m-blocks
TT = 512           # t-tile (free dim of the big matmuls)
NT = T // TT       # 8  t-tiles
TS = TT // P       # 4  t-subblocks per t-tile
NTS = T // P       # 32 t-subblocks total
NH = T // 256      # 16 emb staging half-tiles
EH = E // 512      # 2  e-halves (psum free-dim limit for fp32)
SCALE = 1.0 / float(np.sqrt(E))

# norm column split: [ACT | gpsimd | DVE]
NSPLIT = (256, 640, 128)

REPS = int(os.environ.get("KERNEL_REPS", "1"))

_cache = {}
LAST_RUN = {}


def _bcast_ap(ap, p):
    """[free...] DRAM AP -> [p, free...] partition-broadcast AP."""
    import concourse.bass as bass

    return bass.AP(tensor=ap.tensor, offset=ap.offset, ap=[[0, p], *ap.ap])


def _build(apply_affine, reps=None):
    if reps is None:
        reps = REPS
    import concourse.bacc as bacc
    import concourse.mybir as mybir
    import concourse.tile as tile
    from concourse.masks import make_identity

    f32 = mybir.dt.float32
    bf16 = mybir.dt.bfloat16
    AF = mybir.ActivationFunctionType
    OP = mybir.AluOpType
    nc = bacc.Bacc(None, target_bir_lowering=False)

    emb = nc.dram_tensor("emb", (T, E), f32, kind="ExternalInput")
    latent = nc.dram_tensor("latent", (M, L), f32, kind="ExternalInput")
    Wq = nc.dram_tensor("Wq", (E, E), f32, kind="ExternalInput")
    bq = nc.dram_tensor("bq", (E,), f32, kind="ExternalInput")
    Wk = nc.dram_tensor("Wk", (L, E), f32, kind="ExternalInput")
    bk = nc.dram_tensor("bk", (E,), f32, kind="ExternalInput")
    Wv = nc.dram_tensor("Wv", (L, E), f32, kind="ExternalInput")
    bv = nc.dram_tensor("bv", (E,), f32, kind="ExternalInput")
    gamma = nc.dram_tensor("gamma", (E,), f32, kind="ExternalInput")
    beta = nc.dram_tensor("beta", (E,), f32, kind="ExternalInput")
    out = nc.dram_tensor("out", (T, E), f32, kind="ExternalOutput")

    with tile.TileContext(nc) as tc, \
         tc.tile_pool(name="const", bufs=1) as const, \
         tc.tile_pool(name="persist", bufs=1) as persist, \
         tc.tile_pool(name="wload", bufs=2) as wload, \
         tc.tile_pool(name="embtp", bufs=2) as embtp, \
         tc.tile_pool(name="xsb", bufs=4) as xsbp, \
         tc.tile_pool(name="xout", bufs=2) as xoutp, \
         tc.tile_pool(name="small", bufs=6) as small, \
         tc.tile_pool(name="sqscr", bufs=1) as sqscr, \
         tc.tile_pool(name="psum_tr", bufs=2, space="PSUM") as psum_tr, \
         tc.tile_pool(name="psum_mm", bufs=2, space="PSUM") as psum_mm, \
         tc.tile_pool(name="psum_p2", bufs=2, space="PSUM") as psum_p2:

        # ---- constants ----
        f32r = mybir.dt.float32r
        ident = const.tile([P, P], bf16)
        make_identity(nc, ident)
        bq_bf = const.tile([P, ES], bf16)
        nc.gpsimd.dma_start(out=bq_bf, in_=bq[:].rearrange("(o p) -> p o", p=P))
        bk_pp = const.tile([P, ES], f32)
        nc.sync.dma_start(bk_pp, bk[:].rearrange("(o p) -> p o", p=P))
        bv_bc = const.tile([P, E], f32)
        nc.gpsimd.dma_start(out=bv_bc, in_=_bcast_ap(bv[:], P))
        if apply_affine:
            gamma_bc = const.tile([P, E], f32)
            nc.gpsimd.dma_start(out=gamma_bc, in_=_bcast_ap(gamma[:], P))
            beta_bc = const.tile([P, E], f32)
            nc.gpsimd.dma_start(out=beta_bc, in_=_bcast_ap(beta[:], P))
        eps_c = const.tile([P, 1], f32)
        nc.vector.memset(eps_c, EPS)

        for _rep in range(reps):
            # ---- SWDGE cast-DMAs: latent, Wk, Wv ----
            lat_bf = persist.tile([P, MB, L], bf16, tag="latbf")
            nc.gpsimd.dma_start(
                out=lat_bf, in_=latent[:, :].rearrange("(mb p) l -> p mb l", p=P))
            wk_bf = persist.tile([P, LS, E], bf16, tag="wkbf")
            nc.gpsimd.dma_start(
                out=wk_bf, in_=Wk[:, :].rearrange("(lo p) e -> p lo e", p=P))

            # ---- sync HWDGE: Wq f32 (used once, transposed in f32) ----
            wq_f = persist.tile([P, ES, E], f32, tag="wqf32")
            nc.sync.dma_start(
                wq_f, Wq[:, :].rearrange("(eb p) e -> p eb e", p=P))

            # ---- emb: SWDGE cast-DMA into the bf16 stash, one tile/512 t ----
            emb_bf = persist.tile([P, NTS, E], bf16, tag="embbf")

            def emb_tile_dma(it):
                nc.gpsimd.dma_start(
                    out=emb_bf[:, it * TS:(it + 1) * TS, :],
                    in_=emb[it * TT:(it + 1) * TT, :].rearrange(
                        "(ts p) e -> p ts e", p=P))

            emb_tile_dma(0)
            emb_tile_dma(1)
            wv_bf = persist.tile([P, LS, E], bf16, tag="wvbf")
            nc.gpsimd.dma_start(
                out=wv_bf, in_=Wv[:, :].rearrange("(lo p) e -> p lo e", p=P))
            for it in range(2, NT):
                emb_tile_dma(it)

            # ---- latent^T  [l, m] ----
            latT = persist.tile([P, LS, M], bf16, tag="latT")
            for mb in range(MB):
                pst = psum_tr.tile([P, TT], bf16, tag="tr")
                for ls in range(LS):
                    nc.tensor.transpose(pst[:, ls * P:(ls + 1) * P],
                                        lat_bf[:, mb, ls * P:(ls + 1) * P], ident)
                nc.vector.tensor_copy(
                    out=latT[:, :, mb * P:(mb + 1) * P],
                    in_=pst.rearrange("p (ls m) -> p ls m", ls=LS))

            # ---- k^T [e, m] = Wk^T latT (+bk), bf16 ----
            k_bf = persist.tile([P, ES, M], bf16, tag="kbf")
            for es in range(ES):
                ps = psum_mm.tile([P, 512], f32, tag="mm")
                for ls in range(LS):
                    nc.tensor.matmul(ps[:, :M], wk_bf[:, ls, es * P:(es + 1) * P],
                                     latT[:, ls, :],
                                     start=(ls == 0), stop=(ls == LS - 1))
                nc.scalar.activation(k_bf[:, es, :], ps[:, :M], AF.Identity,
                                     bias=bk_pp[:, es:es + 1])

            # ---- c [m] = (bq . k^T) * SCALE  (f32r) ----
            c_pp = const.tile([P, MB], f32)
            for mb in range(MB):
                ps = psum_mm.tile([P, 512], f32, tag="mm")
                for es in range(ES):
                    nc.tensor.matmul(
                        ps[:, :1],
                        k_bf[:, es, mb * P:(mb + 1) * P],
                        bq_bf[:, es:es + 1],
                        start=(es == 0), stop=(es == ES - 1))
                nc.scalar.mul(c_pp[:, mb:mb + 1], ps[:, :1], SCALE)

            # ---- kq [e_in, m] = Wq @ k^T (DVE casts Wq rows to bf16) ----
            kq = persist.tile([P, ES, M], bf16, tag="kq")
            for eb in range(ES):
                wqb = wload.tile([P, E], bf16, tag="wqbf")
                nc.vector.tensor_copy(out=wqb, in_=wq_f[:, eb, :])
                wqT_row = wload.tile([P, ES, P], bf16, tag="wqTrow")
                for half in range(2):
                    pst = psum_tr.tile([P, TT], bf16, tag="tr")
                    for j in range(4):
                        fs = half * 4 + j
                        nc.tensor.transpose(pst[:, j * P:(j + 1) * P],
                                            wqb[:, fs * P:(fs + 1) * P], ident)
                    nc.vector.tensor_copy(
                        out=wqT_row[:, half * 4:(half + 1) * 4, :],
                        in_=pst.rearrange("p (j m) -> p j m", j=4))
                ps = psum_mm.tile([P, 512], f32, tag="mm")
                for fs in range(ES):
                    nc.tensor.matmul(ps[:, :M], wqT_row[:, fs, :], k_bf[:, fs, :],
                                     start=(fs == 0), stop=(fs == ES - 1))
                nc.scalar.copy(kq[:, eb, :], ps[:, :M])

            # ---- v [m, e] = latT^T Wv + bv (unnormalized) ----
            v_bf = persist.tile([P, MB, E], bf16, tag="vbf")
            for mb in range(MB):
                for eh in range(EH):
                    ps = psum_mm.tile([P, 512], f32, tag="mm")
                    for ls in range(LS):
                        nc.tensor.matmul(ps, latT[:, ls, mb * P:(mb + 1) * P],
                                         wv_bf[:, ls, eh * 512:(eh + 1) * 512],
                                         start=(ls == 0), stop=(ls == LS - 1))
                    nc.vector.scalar_tensor_tensor(
                        out=v_bf[:, mb, eh * 512:(eh + 1) * 512],
                        in0=ps, scalar=1.0,
                        in1=bv_bc[:, eh * 512:(eh + 1) * 512],
                        op0=OP.mult, op1=OP.add)

            # ---- pass 1 over T: exp_wT [m, t] and row sums ----
            exp_wT = persist.tile([P, MB, T], bf16, tag="expw")
            s_part = persist.tile([P, MB, NT], f32, tag="spart")
            for it in range(NT):
                embT = embtp.tile([P, ES, TT], bf16, tag="embT")
                for es in range(ES):
                    pst = psum_tr.tile([P, TT], bf16, tag="tr")
                    for ts in range(TS):
                        nc.tensor.transpose(
                            pst[:, ts * P:(ts + 1) * P],
                            emb_bf[:, it * TS + ts, es * P:(es + 1) * P], ident)
                    if es % 4 == 3:
                        nc.scalar.copy(embT[:, es, :], pst)
                    else:
                        nc.vector.tensor_copy(out=embT[:, es, :], in_=pst)
                for mb in range(MB):
                    psw = psum_mm.tile([P, 512], f32, tag="mm")
                    for es in range(ES):
                        nc.tensor.matmul(psw, kq[:, es, mb * P:(mb + 1) * P],
                                         embT[:, es, :],
                                         start=(es == 0), stop=(es == ES - 1))
                    nc.scalar.activation(exp_wT[:, mb, it * TT:(it + 1) * TT], psw,
                                         AF.Exp, bias=c_pp[:, mb:mb + 1], scale=SCALE,
                                         accum_out=s_part[:, mb, it:it + 1])

            # ---- softmax denominators and normalized v ----
            s_tot = small.tile([P, MB, 1], f32, tag="stot")
            nc.vector.reduce_sum(s_tot, s_part, axis=mybir.AxisListType.X)
            inv_s = small.tile([P, MB, 1], f32, tag="invs")
            nc.vector.reciprocal(inv_s, s_tot)
            v_norm = persist.tile([P, MB, E], bf16, tag="vnorm")
            for mb in range(MB):
                nc.vector.tensor_scalar_mul(v_norm[:, mb, :], v_bf[:, mb, :],
                                            inv_s[:, mb, :])

            # ---- pass 2 over T: pred + residual + LayerNorm, fused ----
            n0, n1, n2 = NSPLIT
            for tsb in range(NTS):
                t0 = tsb * P
                psp = psum_p2.tile([P, 1024], f32, tag="p2")
                for eh in range(EH):
                    sl = psp[:, eh * 512:(eh + 1) * 512]
                    for mb in range(MB):
                        nc.tensor.matmul(sl, exp_wT[:, mb, t0:t0 + P],
                                         v_norm[:, mb, eh * 512:(eh + 1) * 512],
                                         start=(mb == 0), stop=False)
                    nc.tensor.matmul(sl, ident,
                                     emb_bf[:, tsb, eh * 512:(eh + 1) * 512],
                                     start=False, stop=True)
                # evacuate x (bf16) + sum_e x in one scalar-engine op
                x_sb = xsbp.tile([P, E], bf16, tag="xsb")
                sx = small.tile([P, 1], f32, tag="sx")
                nc.scalar.activation(x_sb, psp, AF.Copy, accum_out=sx)
                # sum_e x^2 on the DVE: out=(x*1)*x with accum=sum
                sqs = sqscr.tile([P, E], bf16, tag="sqscratch")
                ssq = small.tile([P, 1], f32, tag="ssq")
                nc.vector.scalar_tensor_tensor(
                    out=sqs, in0=x_sb, scalar=1.0, in1=x_sb,
                    op0=OP.mult, op1=OP.mult, accum_out=ssq)
                # LN stats, all-DVE chain (no cross-engine hops):
                # nmusq = -(sx*sx)/E^2 ; veps = ssq/E + nmusq ;
                # rstd = 1/sqrt(|veps|+eps) (ACT) ; nmr = -sx*rstd/E
                nmusq = small.tile([P, 1], f32, tag="nmusq")
                nc.gpsimd.tensor_scalar(nmusq, in0=sx, scalar1=sx,
                                        scalar2=-1.0 / (E * E),
                                        op0=OP.mult, op1=OP.mult)
                veps = small.tile([P, 1], f32, tag="veps")
                nc.vector.scalar_tensor_tensor(
                    out=veps, in0=ssq, scalar=1.0 / E, in1=nmusq,
                    op0=OP.mult, op1=OP.add)
                rstd = small.tile([P, 1], f32, tag="rstd")
                nc.scalar.activation(rstd, veps, AF.Abs_reciprocal_sqrt,
                                     bias=eps_c)
                nmr = small.tile([P, 1], f32, tag="nmr")
                nc.vector.tensor_scalar(nmr, in0=sx, scalar1=rstd,
                                        scalar2=-1.0 / E,
                                        op0=OP.mult, op1=OP.mult)
                # out = x*rstd - mu*rstd, columns split ACT / gpsimd / DVE
                xo = xoutp.tile([P, E], f32, tag="xo")
                nc.scalar.activation(xo[:, 0:n0], x_sb[:, 0:n0],
                                     AF.Identity, bias=nmr, scale=rstd)
                nc.gpsimd.tensor_scalar(xo[:, n0:n0 + n1],
                                        in0=x_sb[:, n0:n0 + n1],
                                        scalar1=rstd, scalar2=nmr,
                                        op0=OP.mult, op1=OP.add)
                nc.vector.tensor_scalar(xo[:, n0 + n1:E],
                                        in0=x_sb[:, n0 + n1:E],
                                        scalar1=rstd, scalar2=nmr,
                                        op0=OP.mult, op1=OP.add)
                if apply_affine:
                    nc.vector.tensor_mul(xo, xo, gamma_bc)
                    nc.gpsimd.tensor_add(xo, xo, beta_bc)
                nc.sync.dma_start(out[t0:t0 + P, :], xo)

    nc.compile()
    return nc


def kernel(emb, latent, Wq, bq, Wk, bk, Wv, bv, gamma, beta):
    from concourse.bass_utils import run_bass_kernel_spmd

    gamma = np.ascontiguousarray(gamma, dtype=np.float32)
    beta = np.ascontiguousarray(beta, dtype=np.float32)
    apply_affine = not (np.all(gamma == 1.0) and np.all(beta == 0.0))

    key = ("nc", apply_affine)
    if key not in _cache:
        _cache[key] = _build(apply_affine)
    nc = _cache[key]

    emb = np.ascontiguousarray(emb, dtype=np.float32)
    latent = np.ascontiguousarray(latent, dtype=np.float32)
    shared = {
        "Wq": np.ascontiguousarray(Wq, dtype=np.float32),
        "bq": np.ascontiguousarray(bq, dtype=np.float32),
        "Wk": np.ascontiguousarray(Wk, dtype=np.float32),
        "bk": np.ascontiguousarray(bk, dtype=np.float32),
        "Wv": np.ascontiguousarray(Wv, dtype=np.float32),
        "bv": np.ascontiguousarray(bv, dtype=np.float32),
        "gamma": gamma,
        "beta": beta,
    }
    in_maps = [
        {"emb": emb[b], "latent": latent[b], **shared} for b in range(B)
    ]
    trace = bool(int(os.environ.get("KERNEL_TRACE", "0")))
    res = run_bass_kernel_spmd(nc, in_maps, list(range(B)), trace=trace)
    LAST_RUN["exec_time_ns"] = res.exec_time_ns
    LAST_RUN["mean_exec_time_ns"] = res.mean_exec_time_ns
    LAST_RUN["profile_json"] = res.profile_json
    return np.stack([res.results[b]["out"] for b in range(B)], axis=0)
